# revision 1
# baseline (speedup 1.0000x reference)
"""GatedLinearRecurrence Trainium2 kernel (8-core SPMD, Bass/Tile).

Strategy: shard (batch=2) x (4 sequence chunks of 1024 tokens) across 8 cores.
Each core processes 1152 tokens: a 128-token warm-up window (re-computed
redundantly; the recurrence decay e^{-~100} makes carry-in truncation error
~1e-24) followed by its 1024 "main" tokens.  No collectives needed.

Per-core pipeline (channels-on-partitions, tokens-on-free layout):
  LN(x) [t,d] -> PE-transpose -> x̂T [d,t] -> in_proj (f32r matmul)
  -> causal depthwise conv (4 shifted tensor_scalar ops) -> silu -> mask
  -> gate matmul -> sigmoid -> b=-(1-a)*xc -> tensor_tensor_scan (h=-h)
  -> y*silu(z) -> out_proj -> residual subtract -> out [t,dm].

The sign trick: scan data1 = (a-1)*x_conv = -b gives -h; -h*silu(z) = -yg;
out = x - matmul(-yg) = x + proj(yg).

Scheduling notes: x̂T is stored in 384-column chunk tiles so in_proj can
start before layernorm finishes; PSUM evacuations ride the Scalar engine
(Identity/Copy activations) to unload DVE; the scan is chunked (chained via
`initial=`) and interleaved into the gate loop so out_proj follows with no
PE gap; scan outputs reuse the a-tile pool slots (same tag) to fit SBUF.
"""
import sys

for p in ("/opt/trn_rl_repo", "/root/.axon_site/_ro/trn_rl_repo"):
    if p not in sys.path:
        sys.path.insert(0, p)

import numpy as np

import concourse.bass as bass
import concourse.bacc as bacc
import concourse.tile as tile
import concourse.mybir as mybir
from concourse.bass_utils import run_bass_kernel_spmd
from concourse.masks import make_identity

F32 = mybir.dt.float32
F32R = mybir.dt.float32r
AF = mybir.ActivationFunctionType
OP = mybir.AluOpType

B, L, D = 2, 4096, 1024
DI = 2048            # d_inner
NT = 1152            # tokens per core (128 warm-up + 1024 main)
W = 128              # warm-up tokens
CHUNK = 1024
NTT = NT // 128      # 9 token tiles
KD = D // 128        # 8 k-tiles over d_model
KC = DI // 128       # 16 k-tiles over d_inner
TC = 384             # matmul N chunk (3 per core)
NTC = NT // TC
EPS = 1e-5

_cache = {}


def _build():
    nc = bacc.Bacc(None, target_bir_lowering=False)

    x_h = nc.dram_tensor("x", [NT, D], F32, kind="ExternalInput")
    w1x_h = nc.dram_tensor("w1x", [D, DI], F32, kind="ExternalInput")
    w1z_h = nc.dram_tensor("w1z", [D, DI], F32, kind="ExternalInput")
    gw_h = nc.dram_tensor("gw", [DI, DI], F32, kind="ExternalInput")
    op_h = nc.dram_tensor("opw", [DI, D], F32, kind="ExternalInput")
    convw_h = nc.dram_tensor("convw", [128, KC * 4], F32, kind="ExternalInput")
    convb_h = nc.dram_tensor("convb", [128, KC], F32, kind="ExternalInput")
    gateb_h = nc.dram_tensor("gateb", [128, KC], F32, kind="ExternalInput")
    normb_h = nc.dram_tensor("normb", [128, KD], F32, kind="ExternalInput")
    mask_h = nc.dram_tensor("mask", [1, NT], F32, kind="ExternalInput")
    out_h = nc.dram_tensor("out", [CHUNK, D], F32, kind="ExternalOutput")
    z_h = nc.dram_tensor("z_scratch", [KC, 128, NT], F32, kind="Internal")
    yg_h = nc.dram_tensor("yg_scratch", [KC, 128, CHUNK], F32, kind="Internal")

    with tile.TileContext(nc) as tc:
        with tc.tile_pool(name="consts", bufs=1) as consts:

            ident = consts.tile([128, 128], F32, name="ident")
            make_identity(nc, ident)
            mask_sb = consts.tile([128, W], F32R, name="mask_sb")
            nc.gpsimd.dma_start(
                out=mask_sb,
                in_=bass.AP(tensor=mask_h, offset=0, ap=[[0, 128], [1, W]]).bitcast(F32R),
            )
            convw = consts.tile([128, KC * 4], F32, name="convw")
            nc.gpsimd.dma_start(out=convw, in_=convw_h.ap())
            convb = consts.tile([128, KC], F32, name="convb")
            nc.gpsimd.dma_start(out=convb, in_=convb_h.ap())
            gateb = consts.tile([128, KC], F32, name="gateb")
            nc.gpsimd.dma_start(out=gateb, in_=gateb_h.ap())
            normb = consts.tile([128, KD], F32, name="normb")
            nc.gpsimd.dma_start(out=normb, in_=normb_h.ap())
            eps_t = consts.tile([128, 1], F32, name="eps_t")
            nc.vector.memset(eps_t, EPS)

            with tc.tile_pool(name="xcp", bufs=1) as xcp:
                xc = [xcp.tile([128, NT], F32R, name=f"xct{e}") for e in range(KC)]

                # ---- S1-S3: LN, transpose, in_proj (x & z), conv, silu ----
                with tc.tile_pool(name="xT", bufs=1) as xTp, \
                     tc.tile_pool(name="s1roll", bufs=2) as s1r, \
                     tc.tile_pool(name="stat", bufs=4) as stp, \
                     tc.tile_pool(name="w1s", bufs=3) as ws, \
                     tc.tile_pool(name="zr", bufs=4) as zrp, \
                     tc.tile_pool(name="psmm", bufs=5, space="PSUM") as psmm, \
                     tc.tile_pool(name="pstr", bufs=2, space="PSUM") as pstr:

                    # x-hat-T chunk tiles [d-tile][t-chunk]: finer deps, so
                    # the first in_proj matmuls start after 3 LN iterations.
                    xT = [[xTp.tile([128, TC], F32R, name=f"xTt{d_}_{c_}")
                           for c_ in range(NTC)] for d_ in range(KD)]

                    for it in range(NTT):
                        tc3, col = it // 3, (it % 3) * 128
                        xt = s1r.tile([128, D], F32, tag="xt", bufs=3, name="xt")
                        nc.sync.dma_start(out=xt, in_=x_h.ap()[it * 128:(it + 1) * 128, :])
                        stats = stp.tile([128, 2, 6], F32, tag="stats", name="stats")
                        nc.vector.bn_stats(out=stats[:, 0, :], in_=xt[:, 0:512])
                        nc.vector.bn_stats(out=stats[:, 1, :], in_=xt[:, 512:1024])
                        mv = stp.tile([128, 2], F32, tag="mv", name="mv")
                        nc.vector.bn_aggr(out=mv, in_=stats)
                        rstd = stp.tile([128, 1], F32, tag="rstd", name="rstd")
                        nc.scalar.activation(out=rstd, in_=mv[:, 1:2], func=AF.Sqrt,
                                             bias=eps_t, scale=1.0)
                        nc.vector.reciprocal(out=rstd, in_=rstd)
                        nc.vector.tensor_scalar(out=xt, in0=xt, scalar1=mv[:, 0:1],
                                                scalar2=rstd, op0=OP.subtract, op1=OP.mult)
                        for d_ in range(KD):
                            pst = pstr.tile([128, 128], F32, tag="tr", name="pst")
                            nc.tensor.transpose(pst, xt[:, d_ * 128:(d_ + 1) * 128], ident)
                            # evac + norm_b on the Scalar engine
                            nc.scalar.activation(
                                out=xT[d_][tc3][:, col:col + 128], in_=pst,
                                func=AF.Identity, bias=normb[:, d_:d_ + 1], scale=1.0)

                    # in_proj x-half + conv + silu + warm-up mask.  The first
                    # 4 ets interleave their t-chunks so the PE never waits on
                    # layernorm chunks still in flight.
                    NW = 4
                    order = [(e, c) for c in range(NTC) for e in range(NW)]
                    order += [(e, c) for e in range(NW, KC) for c in range(NTC)]
                    wts, xins = {}, {}

                    def s2_chain(et, tc3):
                        if tc3 == 0:
                            wt = ws.tile([128, KD, 128], F32R, tag="w1",
                                         bufs=6, name=f"wt{et}")
                            nc.sync.dma_start(
                                out=wt,
                                in_=w1x_h.ap()[:, et * 128:(et + 1) * 128]
                                .rearrange("(kt p) e -> p kt e", p=128).bitcast(F32R))
                            wts[et] = wt
                            xin = s1r.tile([128, NT + 3], F32, tag="xin",
                                           bufs=NW + 2, name=f"xin{et}")
                            nc.vector.memset(xin[:, 0:3], 0.0)
                            xins[et] = xin
                        ps = psmm.tile([128, TC], F32, tag="mm", name="ps")
                        for kt in range(KD):
                            nc.tensor.matmul(
                                ps, wts[et][:, kt, :], xT[kt][tc3],
                                start=(kt == 0), stop=(kt == KD - 1))
                        nc.scalar.copy(
                            out=xins[et][:, 3 + tc3 * TC: 3 + (tc3 + 1) * TC], in_=ps)
                        if tc3 == NTC - 1:
                            xin = xins.pop(et)
                            tmp = s1r.tile([128, NT], F32, tag="ctmp", name="ctmp")
                            nc.vector.tensor_scalar_mul(
                                tmp, xin[:, 0:NT], convw[:, et * 4:et * 4 + 1])
                            for k in range(1, 4):
                                nc.vector.scalar_tensor_tensor(
                                    out=tmp, in0=xin[:, k:k + NT],
                                    scalar=convw[:, et * 4 + k:et * 4 + k + 1],
                                    in1=tmp, op0=OP.mult, op1=OP.add)
                            nc.scalar.activation(out=xc[et], in_=tmp, func=AF.Silu,
                                                 bias=convb[:, et:et + 1], scale=1.0)
                            # mask is non-unit only on the warm-up columns
                            nc.vector.tensor_mul(
                                xc[et][:, 0:W], xc[et][:, 0:W], mask_sb)

                    for et, tc3 in order:
                        s2_chain(et, tc3)

                    # in_proj z-half + silu -> HBM scratch
                    for et in range(KC):
                        wt = ws.tile([128, KD, 128], F32R, tag="w1", bufs=6, name="wtz")
                        nc.sync.dma_start(
                            out=wt,
                            in_=w1z_h.ap()[:, et * 128:(et + 1) * 128]
                            .rearrange("(kt p) e -> p kt e", p=128).bitcast(F32R))
                        for tc3 in range(NTC):
                            ps = psmm.tile([128, TC], F32, tag="mm", name="psz")
                            for kt in range(KD):
                                nc.tensor.matmul(
                                    ps, wt[:, kt, :], xT[kt][tc3],
                                    start=(kt == 0), stop=(kt == KD - 1))
                            zroll = zrp.tile([128, TC], F32, tag="zr", name="zroll")
                            nc.scalar.activation(out=zroll, in_=ps, func=AF.Silu)
                            nc.scalar.dma_start(
                                out=z_h.ap()[et, :, tc3 * TC:(tc3 + 1) * TC], in_=zroll)

                # ---- S4-S6: gate matmul, sigmoid, chunked scan, y*silu(z).
                # y is kept in SBUF as chunk tiles whose column slices feed
                # out_proj directly as stationary operands - no HBM roundtrip.
                with tc.tile_pool(name="yp", bufs=1) as yp:
                    ych = [[None] * NTC for _ in range(KC)]
                    with tc.tile_pool(name="gws", bufs=3) as gs, \
                         tc.tile_pool(name="ach", bufs=6) as ayp, \
                         tc.tile_pool(name="s6roll", bufs=6) as s6r, \
                         tc.tile_pool(name="psg", bufs=6, space="PSUM") as psg:

                        for et in range(KC):
                            gt = gs.tile([128, KC, 128], F32R, tag="gw", name="gt")
                            nc.gpsimd.dma_start(
                                out=gt,
                                in_=gw_h.ap()[:, et * 128:(et + 1) * 128]
                                .rearrange("(kt p) e -> p kt e", p=128).bitcast(F32R))
                            zls = []
                            for tc3 in range(NTC):
                                lo = max(tc3 * TC, W)
                                zl = s6r.tile([128, (tc3 + 1) * TC - lo], F32,
                                              tag="zl", name="zl")
                                nc.gpsimd.dma_start(
                                    out=zl, in_=z_h.ap()[et, :, lo:(tc3 + 1) * TC])
                                zls.append(zl)
                            prev_y = None
                            for tc3 in range(NTC):
                                a_t = ayp.tile([128, TC], F32R, tag="ach", name="ach")
                                ps = psg.tile([128, TC], F32, tag="mm", name="psgt")
                                for kt in range(KC):
                                    nc.tensor.matmul(
                                        ps, gt[:, kt, :],
                                        xc[kt][:, tc3 * TC:(tc3 + 1) * TC],
                                        start=(kt == 0), stop=(kt == KC - 1))
                                nc.scalar.activation(
                                    out=a_t, in_=ps,
                                    func=AF.Sigmoid, bias=gateb[:, et:et + 1], scale=1.0)
                                bt = s6r.tile([128, TC], F32, tag="bt", name="bt")
                                nc.vector.scalar_tensor_tensor(
                                    out=bt, in0=a_t, scalar=1.0,
                                    in1=xc[et][:, tc3 * TC:(tc3 + 1) * TC],
                                    op0=OP.subtract, op1=OP.mult)
                                y_t = yp.tile([128, TC], F32R, name=f"y{et}_{tc3}")
                                init = 0.0 if tc3 == 0 else prev_y[:, TC - 1:TC]
                                nc.vector.tensor_tensor_scan(
                                    out=y_t, data0=a_t, data1=bt, initial=init,
                                    op0=OP.mult, op1=OP.add)
                                ych[et][tc3] = y_t
                                prev_y = y_t
                            # -yg: multiply after the carry chain is complete
                            for tc3 in range(NTC):
                                lo = max(tc3 * TC, W) - tc3 * TC
                                nc.vector.tensor_mul(
                                    ych[et][tc3][:, lo:TC],
                                    ych[et][tc3][:, lo:TC], zls[tc3])

                    # ---- S7: out_proj + residual.  y chunk slices are the
                    # stationary operands; kt-major accumulation, two tb-half
                    # passes of 8 PSUM banks; opt streamed per (pass, kt). ----
                    NTB = CHUNK // 128

                    def yslice(kt, tb):
                        col = W + tb * 128          # absolute column in [0, NT)
                        tc3, off = col // TC, col % TC
                        return ych[kt][tc3][:, off:off + 128]

                    with tc.tile_pool(name="ops", bufs=8) as opp, \
                         tc.tile_pool(name="s7roll", bufs=6) as s7r, \
                         tc.tile_pool(name="s7res", bufs=8) as s7x, \
                         tc.tile_pool(name="psop", bufs=8, space="PSUM") as psop:
                        for nb in range(2):
                            xres = {}
                            for tb in range(NTB):
                                xres[tb] = s7x.tile([128, 512], F32, tag="xres",
                                                    name=f"xres{tb}")
                                nc.scalar.dma_start(
                                    out=xres[tb],
                                    in_=x_h.ap()[W + tb * 128:W + (tb + 1) * 128,
                                                 nb * 512:(nb + 1) * 512])
                            pss = [psop.tile([128, 512], F32, tag="op",
                                             name=f"pso{tb}") for tb in range(NTB)]
                            for kt in range(KC):
                                opt = opp.tile([128, 512], F32R, tag="opw", name="opt")
                                nc.sync.dma_start(
                                    out=opt,
                                    in_=op_h.ap()[kt * 128:(kt + 1) * 128,
                                                  nb * 512:(nb + 1) * 512]
                                    .bitcast(F32R))
                                for tb in range(NTB):
                                    nc.tensor.matmul(
                                        pss[tb], yslice(kt, tb), opt,
                                        start=(kt == 0), stop=(kt == KC - 1))
                            for tb in range(NTB):
                                oh = s7r.tile([128, 512], F32, tag="oh", name="oh")
                                nc.vector.tensor_sub(oh, xres[tb], pss[tb])
                                nc.sync.dma_start(
                                    out=out_h.ap()[tb * 128:(tb + 1) * 128,
                                                   nb * 512:(nb + 1) * 512],
                                    in_=oh)

    nc.compile()
    return nc


def _prep_host(x, norm_w, norm_b, in_proj_w, conv_w, conv_b, gate_w, gate_b,
               out_proj_w):
    w1 = (in_proj_w * norm_w[None, :]).astype(np.float32)
    w1xT = np.ascontiguousarray(w1[:DI].T)           # [D, DI]
    w1zT = np.ascontiguousarray(w1[DI:].T)           # [D, DI]
    gwT = np.ascontiguousarray(gate_w.T)             # [DI, DI]
    opT = np.ascontiguousarray(out_proj_w.T)         # [DI, D]
    convw_r = np.ascontiguousarray(
        conv_w.reshape(KC, 128, 4).transpose(1, 0, 2).reshape(128, KC * 4))
    convb_r = np.ascontiguousarray(conv_b.reshape(KC, 128).T)
    gateb_r = np.ascontiguousarray(gate_b.reshape(KC, 128).T)
    normb_r = np.ascontiguousarray(norm_b.reshape(KD, 128).T)

    in_maps = []
    for core in range(8):
        b, j = core // 4, core % 4
        xs = np.zeros((NT, D), np.float32)
        start = j * CHUNK - W
        mask = np.ones((1, NT), np.float32)
        if j == 0:
            xs[W:] = x[b, 0:CHUNK]
            mask[0, :W] = 0.0
        else:
            xs[:] = x[b, start:start + NT]
        in_maps.append({
            "x": np.ascontiguousarray(xs), "w1x": w1xT, "w1z": w1zT,
            "gw": gwT, "opw": opT, "convw": convw_r, "convb": convb_r,
            "gateb": gateb_r, "normb": normb_r, "mask": mask,
        })
    return in_maps


def kernel(x, norm_w, norm_b, in_proj_w, conv_w, conv_b, gate_w, gate_b,
           out_proj_w, _trace=False, _collect=None):
    x = np.asarray(x, np.float32)
    if "nc" not in _cache:
        _cache["nc"] = _build()
    nc = _cache["nc"]
    in_maps = _prep_host(
        x, np.asarray(norm_w, np.float32), np.asarray(norm_b, np.float32),
        np.asarray(in_proj_w, np.float32), np.asarray(conv_w, np.float32),
        np.asarray(conv_b, np.float32), np.asarray(gate_w, np.float32),
        np.asarray(gate_b, np.float32), np.asarray(out_proj_w, np.float32))
    res = run_bass_kernel_spmd(nc, in_maps, core_ids=list(range(8)), trace=_trace)
    if _collect is not None:
        _collect.append(res)
    out = np.empty((B, L, D), np.float32)
    for core in range(8):
        b, j = core // 4, core % 4
        out[b, j * CHUNK:(j + 1) * CHUNK] = res.results[core]["out"]
    return out



# revision 16
# speedup vs baseline: 1.1094x; 1.1094x over previous
"""GatedLinearRecurrence Trainium2 kernel (8-core SPMD, Bass/Tile).

Sharding: (batch=2) x (4 sequence chunks of 1024 tokens) across 8 cores, each
with a 128-token redundant warm-up window (recurrence decay makes carry-in
truncation error negligible).  No collectives.

v3: fp8 DoubleRow matmuls + bf16 pipeline.
  - in_proj (x and z halves): 2-term weights W ~= Wh(e4m3) + Wl(e5m2), both
    multiplying x-hat(e4m3), accumulated in the same PSUM group.
  - gate matmul: 1-term e4m3 DoubleRow.
  - out_proj: bf16 moving (y-quantization to fp8 costs too much accuracy).
  - 512-wide moving chunks (DR rhs free = 1024) to keep the PE sequencer off
    the critical path; all DMA issued from the Pool sequencer (cheap DGE
    config) with 2D descriptor-friendly access patterns.
  Measured end-to-end rel err (CoreSim + numpy model): ~0.0136 vs 0.02 gate.

Phase 1 (et-major): LN -> PE-transpose -> x-hatT(e4m3) pair tiles ->
  in_proj x+z DR matmuls -> conv (DVE tree / Pool stt chain, bf16) -> silu ->
  xc(bf16); silu(z) -> zs spilled to DRAM (bf16).
Phase 2 (chunk-major, 512-token chunks): quantize xc->e4m3, gate DR matmul ->
  sigmoid -> b=(1-a)*xc -> scan (carry saved pre-ymul) -> y *= silu(z) ->
  out_proj(bf16, streamed opw) + residual -> out(bf16).
"""
import sys

for p in ("/opt/trn_rl_repo", "/root/.axon_site/_ro/trn_rl_repo"):
    if p not in sys.path:
        sys.path.insert(0, p)

import numpy as np
import ml_dtypes

import concourse.bass as bass
import concourse.bacc as bacc
import concourse.tile as tile
import concourse.mybir as mybir
from concourse.bass_utils import run_bass_kernel_spmd
from concourse.masks import make_identity

F32 = mybir.dt.float32
BF16 = mybir.dt.bfloat16
E4 = mybir.dt.float8e4
E5 = mybir.dt.float8e5
AF = mybir.ActivationFunctionType
OP = mybir.AluOpType
MM = mybir.MatmulPerfMode

NE4 = ml_dtypes.float8_e4m3
NE5 = ml_dtypes.float8_e5m2
NBF = ml_dtypes.bfloat16

B, L, D = 2, 4096, 1024
DI = 2048
NT = 1152            # tokens per core (128 warm-up + 1024 main)
W = 128              # warm-up tokens
CHUNK = 1024         # main tokens per core
KD = D // 128        # 8 k-tiles over d_model
KC = DI // 128       # 16 k-tiles over d_inner
CH = 512             # phase-2 chunk width
CHUNKS = [(0, 512), (512, 1024), (1024, 1152)]
NTT = NT // 128      # 9 token tiles
EPS = 1e-5

_cache = {}


def _build():
    nc = bacc.Bacc(None, target_bir_lowering=False)

    x_h = nc.dram_tensor("x", [NT, D], BF16, kind="ExternalInput")
    # per-et-packed weights: row et*128+p, col kt*128+e (DMA'd 4 ets at a go)
    w1xh_h = nc.dram_tensor("w1xh", [DI, D], E4, kind="ExternalInput")
    w1xl_h = nc.dram_tensor("w1xl", [DI, D], E5, kind="ExternalInput")
    w1zh_h = nc.dram_tensor("w1zh", [DI, D], E4, kind="ExternalInput")
    w1zl_h = nc.dram_tensor("w1zl", [DI, D], E5, kind="ExternalInput")
    # gate weights partition-packed [128, KC*DI]
    gw_h = nc.dram_tensor("gw", [128, KC * DI], E4, kind="ExternalInput")
    # out_proj partition-packed [128, KC*D]
    opw_h = nc.dram_tensor("opw", [128, KC * D], BF16, kind="ExternalInput")
    convw_h = nc.dram_tensor("convw", [128, KC * 4], F32, kind="ExternalInput")
    convb_h = nc.dram_tensor("convb", [128, KC], F32, kind="ExternalInput")
    gateb_h = nc.dram_tensor("gateb", [128, KC], F32, kind="ExternalInput")
    normb_h = nc.dram_tensor("normb", [128, KD], F32, kind="ExternalInput")
    mask_h = nc.dram_tensor("mask", [1, W], BF16, kind="ExternalInput")
    out_h = nc.dram_tensor("out", [CHUNK, D], BF16, kind="ExternalOutput")

    with tile.TileContext(nc) as tc:
        with tc.tile_pool(name="consts", bufs=1) as consts, \
             tc.tile_pool(name="resid", bufs=1) as res:

            # emission order tuned so the DMA engines serve the LN-critical
            # x tiles and first in_proj weights before the big resident loads
            ident = consts.tile([128, 128], BF16, name="ident")
            eps_t = consts.tile([128, 1], F32, name="eps_t")
            mask_sb = consts.tile([128, W], BF16, name="mask_sb")
            convw = consts.tile([128, KC * 4], F32, name="convw")
            convb = consts.tile([128, KC], F32, name="convb")
            gateb = consts.tile([128, KC], F32, name="gateb")
            normb = consts.tile([128, KD], F32, name="normb")
            gw2 = res.tile([128, KC * DI], E4, name="gw")
            gw = gw2[:, :].rearrange("p (k e) -> p k e", k=KC)
            opw2 = res.tile([128, KC * D], BF16, name="opw")
            opw = opw2[:, :].rearrange("p (k d) -> p k d", k=KC)
            xc = [res.tile([128, NT], BF16, name=f"xc{e}") for e in range(KC)]
            zs = res.tile([128, KC, CHUNK], BF16, name="zs")

            # ---- Phase 1: LN, transpose, in_proj x+z, conv, silu ----
            with tc.tile_pool(name="s1roll", bufs=2) as s1r, \
                 tc.tile_pool(name="stat", bufs=4) as stp, \
                 tc.tile_pool(name="w1s", bufs=3) as ws, \
                 tc.tile_pool(name="ctmp", bufs=2) as ctp, \
                 tc.tile_pool(name="xT8p", bufs=1) as xT8p, \
                 tc.tile_pool(name="psmm", bufs=4, space="PSUM") as psmm, \
                 tc.tile_pool(name="pstr", bufs=3, space="PSUM") as pstr:

                # x-hatT fp8 pair tiles [d-pair][chunk] (phase-1 lifetime)
                xT8 = [[xT8p.tile([128, 2, c1 - c0], E4, name=f"xT8_{g}_{ci}")
                        for ci, (c0, c1) in enumerate(CHUNKS)]
                       for g in range(KD // 2)]

                wts, xins, zwts = {}, {}, {}
                xts = {}
                for it in range(5):
                    xt = s1r.tile([128, D], BF16, tag="xt", bufs=5, name="xt")
                    nc.sync.dma_start(
                        out=xt, in_=x_h.ap()[it * 128:(it + 1) * 128, :])
                    xts[it] = xt
                make_identity(nc, ident)
                nc.vector.memset(eps_t, EPS)
                nc.gpsimd.dma_start(out=normb, in_=normb_h.ap())

                def load_w1_group(src_h, src_l, e0, n):
                    # one DMA for n consecutive ets: tile [128, n, KD*128]
                    wh = ws.tile([128, 4, KD * 128], E4, tag="w1h", name="wh")
                    nc.gpsimd.dma_start(
                        out=wh[:, 0:n, :],
                        in_=src_h.ap()[e0 * 128:(e0 + n) * 128, :]
                        .rearrange("(g p) e -> p g e", p=128))
                    wl = ws.tile([128, 4, KD * 128], E5, tag="w1l", name="wl")
                    nc.gpsimd.dma_start(
                        out=wl[:, 0:n, :],
                        in_=src_l.ap()[e0 * 128:(e0 + n) * 128, :]
                        .rearrange("(g p) e -> p g e", p=128))
                    return wh, wl

                wts[0] = load_w1_group(w1xh_h, w1xl_h, 0, 4)
                wts[1] = load_w1_group(w1xh_h, w1xl_h, 4, 4)
                zwts[0] = load_w1_group(w1zh_h, w1zl_h, 0, 4)
                nc.gpsimd.dma_start(out=convw, in_=convw_h.ap())
                nc.gpsimd.dma_start(out=convb, in_=convb_h.ap())
                nc.gpsimd.dma_start(out=gateb, in_=gateb_h.ap())
                nc.gpsimd.dma_start(
                    out=mask_sb,
                    in_=bass.AP(tensor=mask_h, offset=0, ap=[[0, 128], [1, W]]))
                for i in range(8):
                    sl = (KC * DI) // 8
                    nc.sync.dma_start(out=gw2[:, i * sl:(i + 1) * sl],
                                      in_=gw_h.ap()[:, i * sl:(i + 1) * sl])

                for it in range(NTT):
                    if it >= 5:
                        xt = s1r.tile([128, D], BF16, tag="xt", bufs=5,
                                      name="xt")
                        nc.sync.dma_start(
                            out=xt, in_=x_h.ap()[it * 128:(it + 1) * 128, :])
                        xts[it] = xt
                    xt = xts[it]
                    stats = stp.tile([128, 2, 6], F32, tag="stats", name="stats")
                    nc.vector.bn_stats(out=stats[:, 0, :], in_=xt[:, 0:512])
                    nc.vector.bn_stats(out=stats[:, 1, :], in_=xt[:, 512:1024])
                    mv = stp.tile([128, 2], F32, tag="mv", name="mv")
                    nc.vector.bn_aggr(out=mv, in_=stats)
                    rstd = stp.tile([128, 1], F32, tag="rstd", name="rstd")
                    nc.scalar.activation(out=rstd, in_=mv[:, 1:2], func=AF.Sqrt,
                                         bias=eps_t, scale=1.0)
                    nc.vector.reciprocal(out=rstd, in_=rstd)
                    nc.vector.tensor_scalar(out=xt, in0=xt, scalar1=mv[:, 0:1],
                                            scalar2=rstd, op0=OP.subtract,
                                            op1=OP.mult)
                    xts[it] = xt
                    # transpose pairs of token tiles as they become available
                    if it % 2 == 1 or it == NTT - 1:
                        tt0 = it - 1 if it % 2 == 1 else it
                        n = 256 if it % 2 == 1 else 128
                        ci = (tt0 * 128) // CH
                        off = (tt0 * 128) % CH
                        for d_ in range(KD):
                            pst = pstr.tile([128, 256], BF16, tag="tr", name="pst")
                            nc.tensor.transpose(
                                pst[:, 0:128],
                                xts[tt0][:, d_ * 128:(d_ + 1) * 128], ident)
                            if n == 256:
                                nc.tensor.transpose(
                                    pst[:, 128:256],
                                    xts[tt0 + 1][:, d_ * 128:(d_ + 1) * 128],
                                    ident)
                            # evac + norm_b, quantize to e4m3
                            nc.scalar.activation(
                                out=xT8[d_ // 2][ci][:, d_ % 2, off:off + n],
                                in_=pst[:, 0:n], func=AF.Identity,
                                bias=normb[:, d_:d_ + 1], scale=1.0)
                        if it % 2 == 1:
                            xts.pop(tt0, None)
                            if it > 1:
                                xts.pop(tt0 - 1, None)

                # in_proj: x then z half per et; first 4 ets interleave chunks
                # so the PE starts as soon as early chunk tiles exist.
                NW = 8
                order = []
                for s in range(3):
                    order += [("x", e, s) for e in range(NW)]
                    order += [("z", e, s) for e in range(4)]
                order += [("xz", e, s) for e in range(NW, KC) for s in range(3)]
                def dr_group(ps, whv, wlv, ci, w):
                    for g in range(KD // 2):
                        nc.tensor.matmul(
                            ps[:, 0:w], whv[:, 2 * g:2 * g + 2, :],
                            xT8[g][ci], start=(g == 0), stop=False,
                            perf_mode=MM.DoubleRow)
                    for g in range(KD // 2):
                        nc.tensor.matmul(
                            ps[:, 0:w], wlv[:, 2 * g:2 * g + 2, :],
                            xT8[g][ci], start=False,
                            stop=(g == KD // 2 - 1), perf_mode=MM.DoubleRow)

                def w1_view(grp, et):
                    wh, wl = grp
                    i = et % 4
                    return (wh[:, i, :].rearrange("p (k e) -> p k e", k=KD),
                            wl[:, i, :].rearrange("p (k e) -> p k e", k=KD))

                def inx_chain(et, s):
                    if s == 0:
                        nxt = et // 4 + 1
                        if nxt < 4 and nxt not in wts:
                            wts[nxt] = load_w1_group(w1xh_h, w1xl_h,
                                                     nxt * 4, 4)
                        xin = s1r.tile([128, NT + 3], BF16, tag="xin",
                                       bufs=NW + 1, name=f"xin{et}")
                        nc.vector.memset(xin[:, 0:3], 0.0)
                        xins[et] = xin
                    c0, c1 = CHUNKS[s]
                    w = c1 - c0
                    ps = psmm.tile([128, 512], F32, tag="mx", name="psx")
                    whv, wlv = w1_view(wts[et // 4], et)
                    dr_group(ps, whv, wlv, s, w)
                    # evacuate segment, f32 -> bf16 (Pool cannot read PSUM)
                    dst = xins[et][:, 3 + c0:3 + c1]
                    if et % 3 == 1:
                        nc.vector.tensor_copy(dst, ps[:, 0:w])
                    else:
                        nc.scalar.activation(out=dst, in_=ps[:, 0:w],
                                             func=AF.Copy)
                    if s == 2:
                        # conv: DVE bf16 ts/tt tree for most ets, Pool stt
                        # chain for the rest
                        xin = xins.pop(et)
                        t1 = ctp.tile([128, NT], BF16, tag="ct1", bufs=2,
                                      name="ct1")
                        t2 = ctp.tile([128, NT], BF16, tag="ct2", bufs=2,
                                      name="ct2")
                        nc.vector.tensor_scalar_mul(
                            t1, xin[:, 0:NT], convw[:, et * 4:et * 4 + 1])
                        nc.vector.tensor_scalar_mul(
                            t2, xin[:, 1:1 + NT],
                            convw[:, et * 4 + 1:et * 4 + 2])
                        nc.vector.tensor_add(t1, t1, t2)
                        nc.vector.tensor_scalar_mul(
                            t2, xin[:, 2:2 + NT],
                            convw[:, et * 4 + 2:et * 4 + 3])
                        nc.vector.tensor_add(t1, t1, t2)
                        nc.vector.tensor_scalar_mul(
                            t2, xin[:, 3:3 + NT],
                            convw[:, et * 4 + 3:et * 4 + 4])
                        nc.vector.tensor_add(t1, t1, t2)
                        nc.scalar.activation(out=xc[et], in_=t1, func=AF.Silu,
                                             bias=convb[:, et:et + 1], scale=1.0)
                        nc.vector.tensor_mul(xc[et][:, 0:W], xc[et][:, 0:W],
                                             mask_sb)

                def inz_chain(et, s):
                    if s == 0:
                        nxt = et // 4 + 1
                        if nxt < 4 and nxt not in zwts:
                            zwts[nxt] = load_w1_group(w1zh_h, w1zl_h,
                                                      nxt * 4, 4)
                    whv, wlv = w1_view(zwts[et // 4], et)
                    c0, c1 = CHUNKS[s]
                    w = c1 - c0
                    ps = psmm.tile([128, 512], F32, tag="mx", name="psz")
                    dr_group(ps, whv, wlv, s, w)
                    # silu evac of main-token part into resident zs
                    lo = max(c0, W)
                    if lo < c1:
                        nc.scalar.activation(
                            out=zs[:, et, lo - W:c1 - W],
                            in_=ps[:, lo - c0:c1 - c0], func=AF.Silu)

                for kind, et, s in order:
                    if kind in ("x", "xz"):
                        inx_chain(et, s)
                    if kind == "z":
                        inz_chain(et, s)
                    if kind == "xz" and s == 2:
                        for sz in range(3):
                            inz_chain(et, sz)
                    elif kind == "z" and et == 3 and s == 2:
                        for ez in range(4, NW):
                            for sz in range(3):
                                inz_chain(ez, sz)

            # ---- Phase 2 (chunk-major): gate, scan, y, out_proj ----
            with tc.tile_pool(name="p2roll", bufs=2) as p2r, \
                 tc.tile_pool(name="ab", bufs=4) as abp, \
                 tc.tile_pool(name="ych", bufs=2) as ycp, \
                 tc.tile_pool(name="ops", bufs=4) as opp, \
                 tc.tile_pool(name="s7roll", bufs=2) as s7r, \
                 tc.tile_pool(name="psg", bufs=4, space="PSUM") as psg, \
                 tc.tile_pool(name="psop", bufs=4, space="PSUM") as psop:

                for i in range(8):
                    sl = (KC * D) // 8
                    nc.sync.dma_start(out=opw2[:, i * sl:(i + 1) * sl],
                                      in_=opw_h.ap()[:, i * sl:(i + 1) * sl])
                ych = [None] * len(CHUNKS)
                carry = [None] * len(CHUNKS)

                def emit_gate(ci):
                    c0, c1 = CHUNKS[ci]
                    w = c1 - c0
                    mo = W if ci == 0 else 0
                    xc8c = p2r.tile([128, KC, CH], E4, tag="xc8", name="xc8")
                    for et in range(KC):
                        nc.gpsimd.tensor_copy(xc8c[:, et, 0:w],
                                              xc[et][:, c0:c1])
                    ych[ci] = ycp.tile([128, KC, CH], BF16, tag="y",
                                       name=f"y{ci}")
                    for et in range(KC):
                        ps = psg.tile([128, CH], F32, tag="g", name="psg")
                        for g in range(KC // 2):
                            nc.tensor.matmul(
                                ps[:, 0:w],
                                gw[:, 2 * g:2 * g + 2, et * 128:(et + 1) * 128],
                                xc8c[:, 2 * g:2 * g + 2, 0:w],
                                start=(g == 0), stop=(g == KC // 2 - 1),
                                perf_mode=MM.DoubleRow)
                        a_t = abp.tile([128, CH], BF16, tag="a", name="a_t")
                        nc.scalar.activation(out=a_t[:, 0:w], in_=ps[:, 0:w],
                                             func=AF.Sigmoid,
                                             bias=gateb[:, et:et + 1], scale=1.0)
                        omb = abp.tile([128, CH], BF16, tag="omb", name="omb")
                        nc.vector.tensor_scalar(out=omb[:, 0:w], in0=a_t[:, 0:w],
                                                scalar1=-1.0, scalar2=1.0,
                                                op0=OP.mult, op1=OP.add)
                        nc.vector.tensor_mul(omb[:, 0:w], omb[:, 0:w],
                                             xc[et][:, c0:c1])
                        init = 0.0 if ci == 0 else carry[ci - 1][:, et, :]
                        nc.vector.tensor_tensor_scan(
                            out=ych[ci][:, et, 0:w], data0=a_t[:, 0:w],
                            data1=omb[:, 0:w], initial=init,
                            op0=OP.mult, op1=OP.add)
                    # save raw carries before ymul clobbers the last column
                    if ci < len(CHUNKS) - 1:
                        carry[ci] = p2r.tile([128, KC, 1], BF16, tag="carry",
                                             bufs=3, name="carry")
                        nc.vector.tensor_copy(carry[ci], ych[ci][:, :, w - 1:w])
                    # y *= silu(z) on main cols (one 3D op)
                    z0 = c0 + mo - W
                    nc.vector.tensor_mul(
                        ych[ci][:, :, mo:w], ych[ci][:, :, mo:w],
                        zs[:, :, z0:z0 + (w - mo)])

                def emit_outproj(ci):
                    c0, c1 = CHUNKS[ci]
                    mo = W if ci == 0 else 0
                    blocks = list(range(c0 + mo, c1, 128))
                    nbl = len(blocks)
                    for nb in range(2):
                        xres = s7r.tile([128, 4, 512], BF16, tag="xres",
                                        name="xres")
                        nc.sync.dma_start(
                            out=xres[:, 0:nbl, :],
                            in_=x_h.ap()[blocks[0]:blocks[0] + nbl * 128,
                                         nb * 512:(nb + 1) * 512]
                            .rearrange("(g p) e -> p g e", p=128))
                        pss = {}
                        for boff in blocks:
                            pss[boff] = psop.tile([128, 512], F32, tag="op",
                                                  name="pso")
                        for kt in range(KC):
                            for boff in blocks:
                                rel = boff - c0
                                nc.tensor.matmul(
                                    pss[boff], ych[ci][:, kt, rel:rel + 128],
                                    opw[:, kt, nb * 512:(nb + 1) * 512],
                                    start=(kt == 0), stop=False)
                        for bi, boff in enumerate(blocks):
                            nc.tensor.matmul(
                                pss[boff], ident, xres[:, bi, :],
                                start=False, stop=True)
                        outr = s7r.tile([128, 4, 512], BF16, tag="outr",
                                        name="outr")
                        for bi, boff in enumerate(blocks):
                            nc.scalar.activation(out=outr[:, bi, :],
                                                 in_=pss[boff], func=AF.Copy)
                        nc.sync.dma_start(
                            out=out_h.ap()[blocks[0] - W:
                                           blocks[0] - W + nbl * 128,
                                           nb * 512:(nb + 1) * 512]
                            .rearrange("(g p) e -> p g e", p=128),
                            in_=outr[:, 0:nbl, :])

                for ci in range(len(CHUNKS)):
                    emit_gate(ci)
                    if ci > 0:
                        emit_outproj(ci - 1)
                emit_outproj(len(CHUNKS) - 1)

    nc.compile()
    return nc


def _prep_host(x, norm_w, norm_b, in_proj_w, conv_w, conv_b, gate_w, gate_b,
               out_proj_w):
    w1 = (in_proj_w * norm_w[None, :]).astype(np.float32)

    def pack_w1(wslice):
        """[DI, D] torch-layout -> per-et packed hi/lo: row et*128+p, col kt*128+e."""
        wt = wslice.T  # [D, DI]
        hi = wt.astype(NE4)
        lo = (wt - hi.astype(np.float32)).astype(NE5)

        def pack(a):
            r = a.reshape(KD, 128, KC, 128)      # [kt, p, et, e]
            r = r.transpose(2, 1, 0, 3)          # [et, p, kt, e]
            return np.ascontiguousarray(r.reshape(KC * 128, KD * 128))
        return pack(hi), pack(lo)

    w1xh, w1xl = pack_w1(w1[:DI])
    w1zh, w1zl = pack_w1(w1[DI:])

    gwT = gate_w.T.astype(np.float32)            # [DI(c), DI(e)]
    gw8 = gwT.astype(NE4)
    gw_r = np.ascontiguousarray(
        gw8.reshape(KC, 128, DI).transpose(1, 0, 2).reshape(128, KC * DI))
    opT = out_proj_w.T.astype(NBF)                           # [DI, D]
    opw_r = np.ascontiguousarray(
        opT.reshape(KC, 128, D).transpose(1, 0, 2).reshape(128, KC * D))

    convw_r = np.ascontiguousarray(
        conv_w.reshape(KC, 128, 4).transpose(1, 0, 2).reshape(128, KC * 4)
        .astype(np.float32))
    convb_r = np.ascontiguousarray(conv_b.reshape(KC, 128).T.astype(np.float32))
    gateb_r = np.ascontiguousarray(gate_b.reshape(KC, 128).T.astype(np.float32))
    normb_r = np.ascontiguousarray(norm_b.reshape(KD, 128).T.astype(np.float32))

    in_maps = []
    for core in range(8):
        b, j = core // 4, core % 4
        xs = np.zeros((NT, D), np.float32)
        start = j * CHUNK - W
        mask = np.ones((1, W), np.float32)
        if j == 0:
            xs[W:] = x[b, 0:CHUNK]
            mask[0, :] = 0.0
        else:
            xs[:] = x[b, start:start + NT]
        in_maps.append({
            "x": np.ascontiguousarray(xs.astype(NBF)),
            "w1xh": w1xh, "w1xl": w1xl, "w1zh": w1zh, "w1zl": w1zl,
            "gw": gw_r, "opw": opw_r, "convw": convw_r, "convb": convb_r,
            "gateb": gateb_r, "normb": normb_r,
            "mask": mask.astype(NBF),
        })
    return in_maps


def kernel(x, norm_w, norm_b, in_proj_w, conv_w, conv_b, gate_w, gate_b,
           out_proj_w, _trace=False, _collect=None):
    x = np.asarray(x, np.float32)
    if "nc" not in _cache:
        _cache["nc"] = _build()
    nc = _cache["nc"]
    in_maps = _prep_host(
        x, np.asarray(norm_w, np.float32), np.asarray(norm_b, np.float32),
        np.asarray(in_proj_w, np.float32), np.asarray(conv_w, np.float32),
        np.asarray(conv_b, np.float32), np.asarray(gate_w, np.float32),
        np.asarray(gate_b, np.float32), np.asarray(out_proj_w, np.float32))
    res = run_bass_kernel_spmd(nc, in_maps, core_ids=list(range(8)), trace=_trace)
    if _collect is not None:
        _collect.append(res)
    out = np.empty((B, L, D), np.float32)
    for core in range(8):
        b, j = core // 4, core % 4
        out[b, j * CHUNK:(j + 1) * CHUNK] = \
            res.results[core]["out"].astype(np.float32)
    return out
